# revision 61
# baseline (speedup 1.0000x reference)
"""Trainium2 Bass kernel for the DependencyParser biaffine arc scorer.

scores[b,i,j] = W2 @ tanh(Wa@X[b,i] + Wb@X[b,j] + b1) + b2

Shapes (hardcoded): X [32, 96, 512], W1 [512, 1024], b1 [512],
W2 [1, 512], b2 [1].  Output [32, 96, 96] fp32.

Sharding: data-parallel over batch B=32 -> 4 batches per core x 8 cores,
weights replicated.

Per-core schedule (k on partitions, 4 chunks of 128):
  1. PE: Ha/Hb = Wa@X^T, Wb@X^T for all 4 local batches at once, bf16
     (moving operand packs (batch, i) -> N=384 columns).
  2. DVE: fold b1 into Ha while writing each Ha value twice into a
     bf16 "pair" tile (adjacent duplicates, 4B-aligned).
  3. DVE: ONE tensor_tensor add per (batch, kc) builds the whole
     [128, 96*96] tanh preactivation at DVE 2x mode: the Ha operand
     reads the pair tile through a (i:+2)(jp:0)(pair:+1) access
     pattern, so every 32-bit read is a packed identical bf16 pair --
     no broadcast materialization pass needed.
  4. ACT: tanh over [128, 9216] tiles (the engine-busy floor: ~127us
     of the ~154us kernel; tanh is ACT-only at 1 elem/lane/cycle).
  5. PE: W2 contraction as M=1 matmuls col-tiled via tile_position so
     4 groups of 384 cols land on psum partitions 0/32/64/96; psum
     accumulation groups stay open across the kc loop.
  6. DVE copies scores psum->sbuf; DMA to DRAM.
"""

import numpy as np
import ml_dtypes

B, N, H = 32, 96, 512
NCORES = 8
BPC = B // NCORES          # batches per core
P = 128                    # partitions
NKC = H // P               # 4 k-chunks
NHC = H // P               # 4 h-chunks
NB4 = BPC * N              # 384 = batched moving cols
IB = 48                    # i-block size
NIB = N // IB              # 2 i-blocks per batch
FBLK = IB * N              # 4608 free elems per (kc, iblock)
NG = FBLK // 384           # 12 matmul groups of 384 cols per iblock

_CACHE = {}


def _build():
    """Build + compile the per-core Bass module (same program on all cores)."""
    import concourse.bass as bass
    import concourse.mybir as mybir
    import concourse.tile as tile
    from concourse import bacc

    f32 = mybir.dt.float32
    f32r = mybir.dt.float32r
    bf16 = mybir.dt.bfloat16
    i32 = mybir.dt.int32
    Tanh = mybir.ActivationFunctionType.Tanh

    nc = bacc.Bacc("TRN2", target_bir_lowering=False, debug=False)

    xt_d = nc.dram_tensor("xt", [P, NHC * NB4], bf16, kind="ExternalInput")
    wat_d = nc.dram_tensor("wat", [P, NKC * H], bf16, kind="ExternalInput")
    wbt_d = nc.dram_tensor("wbt", [P, NKC * H], bf16, kind="ExternalInput")
    b1_d = nc.dram_tensor("b1c", [P, NKC], f32, kind="ExternalInput")
    w2_d = nc.dram_tensor("w2c", [P, NKC], bf16, kind="ExternalInput")
    sc_d = nc.dram_tensor("scores", [BPC, N * N], f32, kind="ExternalOutput")

    with tile.TileContext(nc) as tc:
        with (
            tc.tile_pool(name="const", bufs=1) as cpool,
            tc.tile_pool(name="t1", bufs=5) as t1pool,
            tc.tile_pool(name="ttan", bufs=4) as ttanpool,
            tc.tile_pool(name="scout", bufs=4) as scpool,
            tc.tile_pool(name="psum_h", bufs=1, space="PSUM") as psum_h,
            tc.tile_pool(name="psum_s", bufs=1, space="PSUM") as psum_sp,
        ):
            # ---- constants ----
            wat_s = cpool.tile([P, NHC * H], bf16, tag="wat")
            wbt_s = cpool.tile([P, NHC * H], bf16, tag="wbt")
            xt_s = cpool.tile([P, NHC * NB4], bf16, tag="xt")
            # host pre-lays-out everything in SBUF layout: fully
            # contiguous DMAs fan out across HW-DGE queues. kc=0 weight
            # blocks and X^T first (the kc0 matmul critical path).
            nc.sync.dma_start(xt_s[:], xt_d[:])
            nc.sync.dma_start(wat_s[:, 0:H], wat_d[:, 0:H])
            nc.sync.dma_start(wbt_s[:, 0:H], wbt_d[:, 0:H])
            nc.sync.dma_start(wat_s[:, H:], wat_d[:, H:])
            nc.sync.dma_start(wbt_s[:, H:], wbt_d[:, H:])
            b1_s = cpool.tile([P, NKC], f32, tag="b1")
            nc.gpsimd.dma_start(b1_s[:], b1_d[:])
            w2_s = cpool.tile([P, NKC], bf16, tag="w2")
            nc.gpsimd.dma_start(w2_s[:], w2_d[:])
            # warm up the ACT tanh table while DMAs/matmuls run
            warm = cpool.tile([P, 1], f32, tag="warm")
            nc.vector.memset(warm[:], 0.0)
            nc.scalar.activation(warm[:], warm[:], Tanh)

            # ---- Ha/Hb for all batches; fold b1; pack Ha pairs ----
            hb_s = cpool.tile([P, NKC * NB4], bf16, tag="hb_s")
            happ = cpool.tile([P, NKC * NB4 * 2], bf16, tag="happ")
            NGB = (N * N) // 384          # 24 groups of 384 cols per batch
            NT = NGB // 4                 # 6 psum tiles of 4 row-groups

            def emit_prep(kc, splits=((0, NB4),)):
                ps_a = psum_h.tile([P, NB4], f32, tag="ha", name=f"ps_a{kc}")
                ps_b = psum_h.tile([P, NB4], f32, tag="hb", name=f"ps_b{kc}")
                hpv = happ[:, kc * NB4 * 2:(kc + 1) * NB4 * 2].rearrange(
                    "p (i two) -> p i two", two=2
                )
                for (c0, cn) in splits:
                    for hc in range(NHC):
                        nc.tensor.matmul(
                            ps_a[:, c0:c0 + cn],
                            wat_s[:, kc * H + hc * P: kc * H + (hc + 1) * P],
                            xt_s[:, hc * NB4 + c0: hc * NB4 + c0 + cn],
                            start=(hc == 0),
                            stop=(hc == NHC - 1),
                        )
                    for hc in range(NHC):
                        nc.tensor.matmul(
                            ps_b[:, c0:c0 + cn],
                            wbt_s[:, kc * H + hc * P: kc * H + (hc + 1) * P],
                            xt_s[:, hc * NB4 + c0: hc * NB4 + c0 + cn],
                            start=(hc == 0),
                            stop=(hc == NHC - 1),
                        )
                    # fold b1; duplicate Ha values into adjacent bf16 pairs
                    nc.vector.tensor_scalar_add(
                        hpv[:, c0:c0 + cn, 0], ps_a[:, c0:c0 + cn],
                        b1_s[:, kc:kc + 1]
                    )
                    nc.vector.tensor_scalar_add(
                        hpv[:, c0:c0 + cn, 1], ps_a[:, c0:c0 + cn],
                        b1_s[:, kc:kc + 1]
                    )
                    nc.vector.tensor_copy(
                        hb_s[:, kc * NB4 + c0: kc * NB4 + c0 + cn],
                        ps_b[:, c0:c0 + cn]
                    )

            def emit_unit(b, kc, ps_list, split):
                # tpre[k,(i,j)] = Ha[k,i] + Hb[k,j] in ONE 2x TT:
                # ha read from the pair tile with innermost (pair: +1, 2)
                # so every 32b read is a packed identical bf16 pair.
                i0 = b * N
                t1 = t1pool.tile([P, N * N], bf16, tag="t1",
                                 name=f"t1_{b}_{kc}")
                ttan = ttanpool.tile([P, N * N], bf16, tag="ttan",
                                     name=f"ttan_{b}_{kc}")
                slices = [(0, 48), (48, 48)] if split else [(0, N)]
                for (si, cnt) in slices:
                    ha4 = happ[:, (kc * NB4 + i0 + si) * 2:
                               (kc * NB4 + i0 + si + cnt) * 2].rearrange(
                        "p (i pair) -> p i pair", pair=2
                    ).unsqueeze(2).broadcast_to([P, cnt, N // 2, 2])
                    hbv = hb_s[:, kc * NB4 + b * N: kc * NB4 + (b + 1) * N]
                    hb4 = hbv.rearrange(
                        "p (jp pair) -> p jp pair", pair=2
                    ).unsqueeze(1).broadcast_to([P, cnt, N // 2, 2])
                    t14 = t1[:, si * N:(si + cnt) * N].rearrange(
                        "p (i jp pair) -> p i jp pair", jp=N // 2, pair=2
                    )
                    nc.vector.tensor_add(t14, hb4, ha4)
                    nc.scalar.activation(
                        ttan[:, si * N:(si + cnt) * N],
                        t1[:, si * N:(si + cnt) * N], Tanh
                    )
                for t in range(NT):
                    for gg in range(4):
                        g = t * 4 + gg
                        nc.tensor.matmul(
                            ps_list[t][32 * gg:32 * gg + 1, :],
                            w2_s[:, kc:kc + 1],
                            ttan[:, g * 384:(g + 1) * 384],
                            start=(kc == 0),
                            stop=(kc == NKC - 1),
                            tile_position=(0, 32 * gg),
                        )

            def alloc_ps(b):
                lst = []
                for t in range(NT):
                    ps_t = psum_sp.tile(
                        [P, 384], f32, tag=f"s{t}", name=f"ps_s{t}_{b}"
                    )
                    lst.append(ps_t)
                return lst

            # prep kc0 then immediately the first main unit, so the first
            # TT/tanh aren't queued behind kc1-3 prep on the DVE stream
            ps0 = alloc_ps(0)
            for kc in range(NKC):
                # kc0: batch-0 columns first so the first TT starts early
                emit_prep(kc, splits=((0, N), (N, NB4 - N)) if kc == 0
                          else ((0, NB4),))
                if kc == 0:
                    emit_unit(0, 0, ps0, split=True)

            # ---- main loop: per batch, kc-interleaved W2 accumulation ----
            for b in range(BPC):
                ps_list = ps0 if b == 0 else alloc_ps(b)
                for kc in range(NKC):
                    if b == 0 and kc == 0:
                        continue
                    emit_unit(b, kc, ps_list,
                              split=(b == BPC - 1 and kc == NKC - 1))
                for t in range(NT):
                    sc_s = scpool.tile([P, 384], f32)
                    nc.vector.tensor_copy(sc_s[:], ps_list[t][:])
                    sc_view = sc_s[:].rearrange("(g r) f -> g r f", r=32)[:, 0, :]
                    nc.sync.dma_start(
                        sc_d[b, t * 1536:(t + 1) * 1536].rearrange(
                            "(g f) -> g f", g=4
                        ),
                        sc_view,
                    )

    nc.compile()
    return nc


def _get_nc():
    if "nc" not in _CACHE:
        _CACHE["nc"] = _build()
    return _CACHE["nc"]


def _make_in_maps(encoded_sequence, W1, b1, W2):
    x = np.asarray(encoded_sequence, dtype=np.float32)
    W1 = np.asarray(W1, dtype=np.float32)
    b1 = np.asarray(b1, dtype=np.float32)
    W2 = np.asarray(W2, dtype=np.float32)

    # weights in SBUF layout [p, (kc, hc, kk)]; X^T in [p, (hc, b, i)]
    def _wlay(w):  # w: [h, k] -> [P, NKC*H]
        a = w.reshape(NHC, P, NKC, P).transpose(1, 2, 0, 3)
        return np.ascontiguousarray(a.reshape(P, NKC * H)).astype(
            ml_dtypes.bfloat16)

    wat = _wlay(W1[:, :H].T)
    wbt = _wlay(W1[:, H:].T)
    b1c = np.ascontiguousarray(b1.reshape(NKC, P).T)  # [128, 4]
    w2c = np.ascontiguousarray(W2[0].reshape(NKC, P).T).astype(ml_dtypes.bfloat16)
    xt = np.ascontiguousarray(x.transpose(0, 2, 1)).astype(ml_dtypes.bfloat16)  # [B, h, n]

    in_maps = []
    for c in range(NCORES):
        xc = xt[c * BPC:(c + 1) * BPC]              # [BPC, h, n]
        xl = xc.reshape(BPC, NHC, P, N).transpose(2, 1, 0, 3)
        in_maps.append({
            "xt": np.ascontiguousarray(xl.reshape(P, NHC * NB4)),
            "wat": wat,
            "wbt": wbt,
            "b1c": b1c,
            "w2c": w2c,
        })
    return in_maps


def kernel(encoded_sequence, W1, b1, W2, b2):
    from concourse import bass_utils

    nc = _get_nc()
    in_maps = _make_in_maps(encoded_sequence, W1, b1, W2)
    res = bass_utils.run_bass_kernel_spmd(nc, in_maps, core_ids=list(range(NCORES)))
    out = np.concatenate(
        [res.results[c]["scores"].reshape(BPC, N, N) for c in range(NCORES)], axis=0
    )
    b2 = np.asarray(b2, dtype=np.float32)
    return (out + b2[0]).astype(np.float32)
